# revision 4
# baseline (speedup 1.0000x reference)
"""GQA kernel for 8 trn2 NeuronCores — v3.

Sharding: tensor-parallel over heads. Core c owns KV head c and Q heads
4c..4c+3 (cols 256c:256c+256 of Wq, col 64c:64c+64 of Wk/Wv, rows
256c:256c+256 of Wo). Each core computes a partial output [B,S,E]
(its ctx slice @ its Wo row-slice); host sums the 8 partials.

Device algorithm (per core, per batch) — v3:
  A. projections:
     Q.T pair tiles [128=2heads x 64d, S] (1/8 scale folded into Wq host-side)
     K.T [64, S] + duplicate to partitions 64:128 (SBUF->SBUF DMA)
     V natural [S, 64] computed directly (lhsT = x.T chunk, rhs = Wv chunk)
       -> vna [128, 16, 65] with ones in col 64 (fused softmax denominator)
  B. per (pair p, q-chunk jq of 512):
       scores S.T[kv, q] row-tiled pairs -> exp on ACT -> pt bf16 [128, 1024]
       ctx NATURAL: ctx[q, 0:65] += pt_chunk.T @ V_aug  (col 64 = denom)
         full 128 output partitions -> 65-wide matmuls (vs 512-wide ctx.T)
       normalize: per-partition reciprocal + tensor_scalar -> ctxn [128,128]
       PE-transpose (identity) ctxn -> ctxT pair tiles [128, S]
  C. out_partial = ctxT.T @ Wo (psum) -> DVE copy -> bf16 staging -> DMA out

All matmuls bf16 / fp32 PSUM. PSUM banks: psc(sc 2x2) + psx(ctx/tp 2) +
psa(proj/out 2) = 8.
"""

import numpy as np
import ml_dtypes

B = 2
S = 2048
E = 2048
HD = 64          # head dim
HPC = 4          # q heads per core
NP = 2           # head pairs per core
QD = HPC * HD    # 256 per-core q dims
NCORES = 8
EC = E // 128    # 16 contraction chunks
NJQ = S // 512   # 4 q-chunks of 512
NKV = S // 128   # 16 kv chunks of 128
BF16 = ml_dtypes.bfloat16

_cache = {}


def _build():
    from contextlib import ExitStack
    from concourse import bacc, tile
    import concourse.mybir as mybir

    bf16 = mybir.dt.bfloat16
    f32 = mybir.dt.float32
    EXP = mybir.ActivationFunctionType.Exp

    nc = bacc.Bacc(
        "TRN2", target_bir_lowering=False, debug=False, num_devices=NCORES)
    qT_d = nc.declare_dram_parameter("qT", [B, E, S], bf16, isOutput=False)
    kT_d = nc.declare_dram_parameter("kT", [B, E, S], bf16, isOutput=False)
    vT_d = nc.declare_dram_parameter("vT", [B, E, S], bf16, isOutput=False)
    wq_d = nc.declare_dram_parameter("wq", [E, QD], bf16, isOutput=False)
    wk_d = nc.declare_dram_parameter("wk", [E, HD], bf16, isOutput=False)
    wv_d = nc.declare_dram_parameter("wv", [E, HD], bf16, isOutput=False)
    wo_d = nc.declare_dram_parameter("wo", [QD, E], bf16, isOutput=False)
    id_d = nc.declare_dram_parameter("ident", [128, 128], bf16, isOutput=False)
    out_d = nc.declare_dram_parameter("out", [B, S, E], bf16, isOutput=True)

    with ExitStack() as ctx:
        tc = ctx.enter_context(tile.TileContext(nc))
        # ---- pools ----
        wpool = ctx.enter_context(tc.tile_pool(name="w", bufs=1))
        qin = ctx.enter_context(tc.tile_pool(name="qin", bufs=16))
        kvin = ctx.enter_context(tc.tile_pool(name="kvin", bufs=4))
        qts = ctx.enter_context(tc.tile_pool(name="qts", bufs=2))
        vnp = ctx.enter_context(tc.tile_pool(name="vnp", bufs=2))
        ptp = ctx.enter_context(tc.tile_pool(name="ptp", bufs=16))
        cnp = ctx.enter_context(tc.tile_pool(name="cnp", bufs=8))
        rcp = ctx.enter_context(tc.tile_pool(name="rcp", bufs=4))
        ostp = ctx.enter_context(tc.tile_pool(name="ostp", bufs=3))
        psa = ctx.enter_context(tc.tile_pool(name="psa", bufs=2, space="PSUM"))
        psx = ctx.enter_context(tc.tile_pool(name="psx", bufs=2, space="PSUM"))
        psc = ctx.enter_context(tc.tile_pool(name="psc", bufs=2, space="PSUM"))

        # ---- weights (loaded once) ----
        wq_sb = wpool.tile([128, EC, QD], bf16)
        nc.sync.dma_start(wq_sb[:], wq_d.rearrange("(c p) m -> p c m", p=128))
        wk_sb = wpool.tile([128, EC, HD], bf16)
        nc.sync.dma_start(wk_sb[:], wk_d.rearrange("(c p) m -> p c m", p=128))
        wv_sb = wpool.tile([128, EC, HD], bf16)
        nc.sync.dma_start(wv_sb[:], wv_d.rearrange("(c p) m -> p c m", p=128))
        wo_sb = wpool.tile([128, 2, E], bf16)
        nc.sync.dma_start(wo_sb[:], wo_d.rearrange("(c p) e -> p c e", p=128))
        id_sb = wpool.tile([128, 128], bf16)
        nc.sync.dma_start(id_sb[:], id_d[:, :])

        def phase_A(b):
            # ---------- Q.T pair tiles [128, S] ----------
            qtiles = []
            for e in range(EC):
                qt = qin.tile([128, S], bf16, tag="qin", name="qt")
                nc.sync.dma_start(qt[:], qT_d[b, e * 128:(e + 1) * 128, :])
                qtiles.append(qt)
            qp_sb = [qts.tile([128, S], bf16, tag=f"qp{p}", name=f"qp{p}")
                     for p in range(NP)]
            for m in range(NP):
                for t in range(NJQ):
                    acc = psa.tile([128, 512], f32, tag="acc", name="qacc")
                    for e in range(EC):
                        nc.tensor.matmul(
                            acc[:], lhsT=wq_sb[:, e, m * 128:(m + 1) * 128],
                            rhs=qtiles[e][:, t * 512:(t + 1) * 512],
                            start=(e == 0), stop=(e == EC - 1))
                    nc.vector.tensor_copy(
                        qp_sb[m][:, t * 512:(t + 1) * 512], acc[:])

            # ---------- K.T [64, S] (+dup), streamed kT ----------
            kt2_sb = qts.tile([128, S], bf16, tag="kt2")
            kaccs = [psa.tile([128, 512], f32, tag="acc", name="kacc")
                     for _ in range(2)]
            for e in range(EC):
                kt_in = kvin.tile([128, S], bf16, tag="kvin", name="ktin")
                nc.sync.dma_start(kt_in[:], kT_d[b, e * 128:(e + 1) * 128, :])
                for t in range(4):
                    r0 = (t % 2) * 64
                    nc.tensor.matmul(
                        kaccs[t // 2][r0:r0 + 64, :], lhsT=wk_sb[:, e, :],
                        rhs=kt_in[:, t * 512:(t + 1) * 512],
                        start=(e == 0), stop=(e == EC - 1),
                        tile_position=(0, r0))
            for t in range(4):
                r0 = (t % 2) * 64
                nc.vector.tensor_copy(
                    kt2_sb[0:64, t * 512:(t + 1) * 512],
                    kaccs[t // 2][r0:r0 + 64, :])
            # duplicate K.T into partitions 64:128 (row-tiled score pairs)
            nc.sync.dma_start(kt2_sb[64:128, :], kt2_sb[0:64, :])

            # ---------- V natural [S, 64] + ones col -> vna [128,16,65] ----
            # NOTE: accumulation regions sharing a PSUM bank must run
            # start..stop strictly sequentially (start zeroes the whole
            # 2KB bank region) -> token-chunk-outer loop, e-inner.
            vna = vnp.tile([128, NKV, HD + 1], bf16, tag="vna", name="vna")
            nc.vector.memset(vna[:, :, HD:HD + 1], 1.0)
            for half in range(2):
                vts = []
                for e in range(EC):
                    vt = qin.tile([128, S // 2], bf16, tag="qin", name="vt")
                    nc.sync.dma_start(
                        vt[:], vT_d[b, e * 128:(e + 1) * 128,
                                    half * 1024:(half + 1) * 1024])
                    vts.append(vt)
                for t8 in range(8):
                    t = half * 8 + t8
                    vacc = psa.tile([128, 64], f32, tag="acc", name="vacc")
                    for e in range(EC):
                        nc.tensor.matmul(
                            vacc[:, 0:64],
                            lhsT=vts[e][:, t8 * 128:(t8 + 1) * 128],
                            rhs=wv_sb[:, e, :],
                            start=(e == 0), stop=(e == EC - 1))
                    nc.vector.tensor_copy(vna[:, t, 0:HD], vacc[:, 0:64])
            return qp_sb, kt2_sb, vna

        def phase_B(b, qp_sb, kt2_sb, vna):
            ctxT_sb = [qts.tile([128, S], bf16, tag=f"ctxT{i}", name=f"ctxT{i}")
                       for i in range(NP)]
            for jq in range(NJQ):
                for p in range(NP):
                    # scores + exp, 2 kv chunks per group
                    pts = [[None] * (NKV // 2) for _ in range(2)]
                    for g in range(NKV // 2):
                        sc_e = psc.tile([128, 1024], f32, tag="sc",
                                        name="sc_e")
                        sc_o = psc.tile([128, 1024], f32, tag="sc",
                                        name="sc_o")
                        for ki in range(2):
                            kv = g * 2 + ki
                            nc.tensor.matmul(
                                sc_e[:, ki * 512:(ki + 1) * 512],
                                lhsT=kt2_sb[0:64, kv * 128:(kv + 1) * 128],
                                rhs=qp_sb[p][0:64, jq * 512:(jq + 1) * 512],
                                start=True, stop=True)
                            nc.tensor.matmul(
                                sc_o[:, ki * 512:(ki + 1) * 512],
                                lhsT=kt2_sb[64:128, kv * 128:(kv + 1) * 128],
                                rhs=qp_sb[p][64:128, jq * 512:(jq + 1) * 512],
                                start=True, stop=True)
                        pt_e = ptp.tile([128, 1024], bf16, tag="pt",
                                        name="pt_e")
                        nc.scalar.activation(pt_e[:], sc_e[:], EXP)
                        pt_o = ptp.tile([128, 1024], bf16, tag="pt",
                                        name="pt_o")
                        nc.scalar.activation(pt_o[:], sc_o[:], EXP)
                        pts[0][g] = pt_e
                        pts[1][g] = pt_o

                    # ctx natural [q, 65] per head, accumulate over kv
                    ctxn = [cnp.tile([128, 128], bf16, tag="ctxn",
                                     name="ctxn") for _ in range(4)]
                    for h in range(2):
                        cps = psx.tile([128, 4 * 65], f32, tag="cps",
                                       name="cps")
                        for j in range(4):
                            for kv in range(NKV):
                                g, ki = kv // 2, kv % 2
                                nc.tensor.matmul(
                                    cps[:, j * 65:(j + 1) * 65],
                                    lhsT=pts[h][g][:, ki * 512 + j * 128:
                                                   ki * 512 + (j + 1) * 128],
                                    rhs=vna[:, kv, :],
                                    start=(kv == 0), stop=(kv == NKV - 1))
                        # normalize: per-partition reciprocal of col 64
                        rc = rcp.tile([128, 4], f32, tag="rc", name="rc")
                        for j in range(4):
                            nc.vector.reciprocal(
                                rc[:, j:j + 1],
                                cps[:, j * 65 + HD:j * 65 + HD + 1])
                        for j in range(4):
                            nc.vector.tensor_scalar_mul(
                                ctxn[j][:, h * 64:(h + 1) * 64],
                                cps[:, j * 65:j * 65 + HD], rc[:, j:j + 1])
                    # PE transpose ctxn [128q,128d] -> ctxT [128d,128q]
                    for j in range(4):
                        tp = psx.tile([128, 128], bf16, tag="cps", name="tp")
                        nc.tensor.transpose(tp[:], ctxn[j][:], id_sb[:])
                        nc.vector.tensor_copy(
                            ctxT_sb[p][:, jq * 512 + j * 128:
                                       jq * 512 + (j + 1) * 128], tp[:])
            return ctxT_sb

        def phase_C(b, ctxT_sb):
            for t in range(S // 128):
                ost = ostp.tile([128, E], bf16, tag="ost", name="ost")
                for eh in range(4):
                    ops = psa.tile([128, 512], f32, tag="acc", name="ops")
                    for kc in range(2):
                        nc.tensor.matmul(
                            ops[:], lhsT=ctxT_sb[kc][:, t * 128:(t + 1) * 128],
                            rhs=wo_sb[:, kc, eh * 512:(eh + 1) * 512],
                            start=(kc == 0), stop=(kc == 1))
                    nc.vector.tensor_copy(ost[:, eh * 512:(eh + 1) * 512],
                                          ops[:])
                nc.sync.dma_start(
                    out_d[b, t * 128:(t + 1) * 128, :], ost[:])

        # software-pipelined emission order: A0 B0 A1 C0 B1 C1
        st0 = phase_A(0)
        ctxT0 = phase_B(0, *st0)
        st1 = phase_A(1)
        phase_C(0, ctxT0)
        ctxT1 = phase_B(1, *st1)
        phase_C(1, ctxT1)
    nc.compile()
    return nc


def _get_nc():
    if "nc" not in _cache:
        _cache["nc"] = _build()
    return _cache["nc"]


def kernel(query, key, value, Wq, Wk, Wv, Wo, _trace=False):
    from concourse.bass_utils import run_bass_kernel_spmd

    def t_bf16(x):
        return np.ascontiguousarray(
            np.asarray(x, np.float32).astype(BF16).transpose(0, 2, 1))

    qT = t_bf16(query)
    kT = t_bf16(key)
    vT = t_bf16(value)
    # fold 1/sqrt(HD) into Wq
    Wq = (np.asarray(Wq, np.float32) * 0.125).astype(BF16)
    Wk = np.asarray(Wk, np.float32).astype(BF16)
    Wv = np.asarray(Wv, np.float32).astype(BF16)
    Wo = np.asarray(Wo, np.float32).astype(BF16)
    ident = np.eye(128, dtype=BF16)

    in_maps = []
    for c in range(NCORES):
        in_maps.append({
            "qT": qT, "kT": kT, "vT": vT,
            "wq": np.ascontiguousarray(Wq[:, c * QD:(c + 1) * QD]),
            "wk": np.ascontiguousarray(Wk[:, c * HD:(c + 1) * HD]),
            "wv": np.ascontiguousarray(Wv[:, c * HD:(c + 1) * HD]),
            "wo": np.ascontiguousarray(Wo[c * QD:(c + 1) * QD, :]),
            "ident": ident,
        })

    nc = _get_nc()
    res = run_bass_kernel_spmd(nc, in_maps, list(range(NCORES)), trace=_trace)
    out = res.results[0]["out"].astype(np.float32)
    for c in range(1, NCORES):
        out += res.results[c]["out"].astype(np.float32)
    if _trace:
        _cache["last_exec_time_ns"] = res.exec_time_ns
        _cache["last_results"] = res
    return out
